# revision 11
# baseline (speedup 1.0000x reference)
"""CharCNN embedding kernel for Trainium2 (8 NeuronCores, Bass/Tile).

Computes out[b,t,f] = sum_k conv_w[f, token_ids[b, t+k-pad], k] with zero
padding outside [0,T) — i.e. one_hot(token_ids) -> Conv1d(V->F, k=3, pad=1).

Strategy: data-parallel over batch (B=8 rows, one per core), weight table
replicated, quantized to int8 with one global symmetric scale (absmax/127).
Accumulation is exact in int16; the device stores int16 and the host
dequantizes (max rel err ~7e-3 vs the 2e-2 gate).

Table rows are fused per token as [A8 | B16 | C8] (512B + 1024B + 512B =
2048B): A/C stay int8, the middle tap is pre-widened to int16 ON HOST so
the device's second add reads 16-bit operands straight out of the gathered
tile — the DVE 2x mode needs all operands 16-bit, and this removes the
engine cast from the dependency chain. DMA is ~12.7MB/core vs ~33MB f32.

Layout: strip layout — partition p owns positions t = p*NT + j, so the +-1
tap shifts are free-dim shifts inside a partition. NT=32 positions split
into 4 tiles of 8 columns.

Device pipeline per tile: DVE op1 part16 = A8' + C8' (int8 adds run 1x;
seam columns at tile borders are small ops reading the neighbor tile or the
host-precomputed strip-edge rows), then DVE op2 part16 += B16 (2x mode).

SWDGE descriptor gen (~0.8us + ~7.6ns/idx, executed by ONE Q7 core pair
selected by queue_num; >1024 idx per gather is a hardware crash; the
sequencer holds each gather until its pair accepts, so queue repeats
head-of-line block) runs on 4 queues = 4 core pairs. Tile 0 is laddered as
single columns behind a tiny dummy so its data lands ~5us earlier.
"""

from contextlib import ExitStack

import numpy as np

import concourse.bacc as bacc
import concourse.bass as bass
import concourse.mybir as mybir
import concourse.tile as tile
from concourse._compat import with_exitstack
from concourse.bass_utils import run_bass_kernel_spmd

B = 8
T = 4096
F = 512
V = 32000
VP = V + 1  # +1 zero row
K = 3
P = 128
NT = T // P  # 32 positions per partition strip
NTILE = 4
J = NT // NTILE  # 8 columns per tile
NQ = 4  # SWDGE queues (Q7 core pairs)
BSW = P // 16  # idx slots per gathered column
SW_TOT = NT * BSW  # idx slots per partition
E = 4 * F  # fused row bytes: A8 (F) + B16 (2F) + C8 (F)
N_CORES = 8
DMA_SCRATCH = 24576

# (tile, col_lo_within_tile, n_cols, queue) in program order; tile 0 single
# columns (first wave q1,q2,q3,q0 behind the dummy that soaks the first
# sequencer hold)
GATHERS = (
    (0, 1, 1, 1), (0, 2, 1, 2), (0, 3, 1, 3), (0, 0, 1, 0),
    (0, 4, 1, 1), (0, 5, 1, 2), (0, 6, 1, 3), (0, 7, 1, 0),
    (1, 0, 4, 1), (1, 4, 4, 2),
    (2, 0, 4, 3), (2, 4, 4, 0),
    (3, 0, 4, 1), (3, 4, 4, 2),
)

_nc_cache = {}


@with_exitstack
def _gather_kernel(ctx: ExitStack, tc: tile.TileContext, out_d, tab_d, idxs_d, bnd_d):
    nc = tc.nc

    idxp = ctx.enter_context(tc.tile_pool(name="idx", bufs=1))
    rp = ctx.enter_context(tc.tile_pool(name="rp", bufs=NTILE))
    pp = ctx.enter_context(tc.tile_pool(name="pp", bufs=NTILE))
    wp = ctx.enter_context(tc.tile_pool(name="wp", bufs=1))

    idxs_t = idxp.tile([P, SW_TOT], mybir.dt.int16)
    nc.sync.dma_start(idxs_t[:], idxs_d[:])
    bnd_t = idxp.tile([P, 2, F], mybir.dt.int8)
    nc.sync.dma_start(bnd_t[:], bnd_d[:])

    R = [None] * NTILE
    for t in range(NTILE):
        R[t] = rp.tile([P, J, E], mybir.dt.int8, tag="R", name=f"R{t}")
    # dummy 16-idx gather: soaks the first-instruction sequencer hold (the
    # first ext-inst occupies the sequencer for its whole generation)
    wdum = wp.tile([P, 1, E], mybir.dt.int8)
    nc.gpsimd.dma_gather(
        wdum[:], tab_d[:], idxs_t[:, 0:1], 16, 16, E, queue_num=0
    )
    for t, cl, ncols, q in GATHERS:
        gcol = t * J + cl
        nc.gpsimd.dma_gather(
            R[t][:, cl : cl + ncols, :],
            tab_d[:],
            idxs_t[:, gcol * BSW : (gcol + ncols) * BSW],
            P * ncols,
            P * ncols,
            E,
            queue_num=q,
        )

    def A8(Rt, lo, hi):
        return Rt[:, lo:hi, 0:F]

    def C8(Rt, lo, hi):
        return Rt[:, lo:hi, 3 * F : 4 * F]

    for t in range(NTILE):
        Rt = R[t]
        B16 = Rt[:].bitcast(mybir.dt.int16)[:, :, F // 2 : F // 2 + F]
        part = pp.tile([P, J, F], mybir.dt.int16, tag="part", name=f"part{t}")
        asrc = bnd_t[:, 0:1, :] if t == 0 else A8(R[t - 1], J - 1, J)
        csrc = bnd_t[:, 1:2, :] if t == NTILE - 1 else C8(R[t + 1], 0, 1)
        if t == 0:
            # laddered column-wise: split ops so DVE starts on the first
            # wave (cols 0-3) before cols 4-7 land; op2 on cols 0..J-2 runs
            # before the seamC column (which needs the next tile's data)
            h = J // 2
            nc.vector.tensor_add(part[:, 0:1, :], asrc, C8(Rt, 1, 2))
            nc.vector.tensor_add(
                part[:, 1 : h - 1, :], A8(Rt, 0, h - 2), C8(Rt, 2, h)
            )
            nc.vector.tensor_add(
                part[:, h - 1 : J - 1, :], A8(Rt, h - 2, J - 2), C8(Rt, h, J)
            )
            nc.vector.tensor_add(
                part[:, 0 : J - 1, :], part[:, 0 : J - 1, :], B16[:, 0 : J - 1, :]
            )
            nc.vector.tensor_add(
                part[:, J - 1 : J, :], A8(Rt, J - 2, J - 1), csrc
            )
            nc.vector.tensor_add(
                part[:, J - 1 : J, :],
                part[:, J - 1 : J, :],
                B16[:, J - 1 : J, :],
            )
            nc.sync.dma_start(out_d[:, t * J : (t + 1) * J, :], part[:])
        else:
            # DVE op1: part[:, j] = A[:, j-1] + C[:, j+1] (int8+int8 -> int16)
            nc.vector.tensor_add(
                part[:, 1 : J - 1, :], A8(Rt, 0, J - 2), C8(Rt, 2, J)
            )
            nc.vector.tensor_add(part[:, 0:1, :], asrc, C8(Rt, 1, 2))
            nc.vector.tensor_add(
                part[:, J - 1 : J, :], A8(Rt, J - 2, J - 1), csrc
            )
            # DVE op2: += B16, all 16-bit -> 2x mode. Last tile split in
            # halves so the final store's DMA drain overlaps the last add.
            if t < NTILE - 1:
                nc.vector.tensor_add(part[:], part[:], B16[:])
                nc.sync.dma_start(out_d[:, t * J : (t + 1) * J, :], part[:])
            else:
                h = J // 2
                nc.vector.tensor_add(
                    part[:, 0:h, :], part[:, 0:h, :], B16[:, 0:h, :]
                )
                nc.sync.dma_start(
                    out_d[:, t * J : t * J + h, :], part[:, 0:h, :]
                )
                nc.vector.tensor_add(
                    part[:, h:J, :], part[:, h:J, :], B16[:, h:J, :]
                )
                nc.sync.dma_start(
                    out_d[:, t * J + h : (t + 1) * J, :], part[:, h:J, :]
                )


def _build_nc():
    if "nc" in _nc_cache:
        return _nc_cache["nc"]
    nc = bacc.Bacc(
        "TRN2",
        target_bir_lowering=False,
        debug=False,
        enable_asserts=False,
        num_devices=N_CORES,
        dynamic_dma_scratch_size=DMA_SCRATCH,
        num_swdge_queues=NQ,
    )
    tab_d = nc.dram_tensor(
        "tab", [VP, E], mybir.dt.int8, kind="ExternalInput"
    ).ap()
    idxs_d = nc.dram_tensor(
        "idxs", [P, SW_TOT], mybir.dt.int16, kind="ExternalInput"
    ).ap()
    bnd_d = nc.dram_tensor(
        "bnd", [P, 2, F], mybir.dt.int8, kind="ExternalInput"
    ).ap()
    out_d = nc.dram_tensor(
        "out", [P, NT, F], mybir.dt.int16, kind="ExternalOutput"
    ).ap()
    with tile.TileContext(nc) as tc:
        _gather_kernel(tc, out_d, tab_d, idxs_d, bnd_d)
    nc.compile()
    _nc_cache["nc"] = nc
    return nc


def _wrap16(stream):
    # gather idx wrap: idx i read from partition i%16, slot i//16; x8 replicas
    n = stream.shape[-1]
    w = stream.reshape(*stream.shape[:-1], n // 16, 16)
    w = np.swapaxes(w, -1, -2)  # [..., 16, n//16]
    reps = [1] * (w.ndim - 2) + [8, 1]
    return np.tile(w, reps)  # [..., 128, n//16]


def _host_prep(token_ids, conv_w):
    # fused row v: [A8 | B16 | C8] with X[k] ~ conv_w[:, v, k] / step
    w = np.asarray(conv_w, dtype=np.float32)
    step = (float(np.abs(w).max()) / 127.0) or 1.0
    q = np.rint(w.transpose(1, 2, 0).reshape(V, K * F) * (1.0 / step))
    q16 = np.clip(q, -127, 127).astype(np.int16)  # [V, 3F] taps A|B|C
    tab = np.zeros((VP, E), dtype=np.int8)
    tab[:V, 0:F] = q16[:, 0:F].astype(np.int8)
    tab[:V, F : 3 * F] = (
        np.ascontiguousarray(q16[:, F : 2 * F]).view(np.int8).reshape(V, 2 * F)
    )
    tab[:V, 3 * F : 4 * F] = q16[:, 2 * F : 3 * F].astype(np.int8)

    tok = np.asarray(token_ids).astype(np.int16)  # V=32000 fits int16
    strip = tok.reshape(B, P, NT)

    # fused streams: per gather, stream[g*128 + p] = strip[b, p, col0+g]
    idxs = np.empty((B, P, SW_TOT), dtype=np.int16)
    for t, cl, ncols, _ in GATHERS:
        gcol = t * J + cl
        x = strip[:, :, gcol : gcol + ncols]  # [b, p, g]
        stream = np.ascontiguousarray(x.transpose(0, 2, 1)).reshape(B, ncols * P)
        idxs[:, :, gcol * BSW : (gcol + ncols) * BSW] = _wrap16(stream)

    # strip-edge rows, host-gathered: bnd[:, 0] = A[tok[p*NT-1]] (zeros at
    # p=0), bnd[:, 1] = C[tok[p*NT+NT]] (zeros at p=127)
    bnd = np.zeros((B, P, 2, F), dtype=np.int8)
    bnd[:, 1:, 0] = tab[:, 0:F][strip[:, :-1, NT - 1]]
    bnd[:, : P - 1, 1] = tab[:, 3 * F : 4 * F][strip[:, 1:, 0]]
    return tab, np.ascontiguousarray(idxs), bnd, step


def kernel(token_ids, conv_w):
    tab, idxs, bnd, step = _host_prep(token_ids, conv_w)
    nc = _build_nc()
    in_maps = [
        {"tab": tab, "idxs": idxs[b], "bnd": bnd[b]} for b in range(B)
    ]
    res = run_bass_kernel_spmd(nc, in_maps, core_ids=list(range(N_CORES)))
    # [P, NT, F] with t = p*NT + j flattens directly to [T, F]
    out = np.stack(
        [res.results[b]["out"].reshape(T, F).astype(np.float32) for b in range(B)],
        axis=0,
    )
    out *= np.float32(step)
    return np.ascontiguousarray(out)


# revision 12
# speedup vs baseline: 1.1671x; 1.1671x over previous
"""CharCNN embedding kernel for Trainium2 (8 NeuronCores, Bass/Tile).

Computes out[b,t,f] = sum_k conv_w[f, token_ids[b, t+k-pad], k] with zero
padding outside [0,T) — i.e. one_hot(token_ids) -> Conv1d(V->F, k=3, pad=1).

Strategy: data-parallel over batch (B=8 rows, one per core), weight table
replicated, quantized to int8 with one global symmetric scale (absmax/127).
Accumulation is exact in int16; the device stores int16 and the host
dequantizes (max rel err ~7e-3 vs the 2e-2 gate). DMA ~10.6MB/core vs ~33MB
for f32. Fused [A|B|C] 1536B rows — the 1.5KB (non-power-of-2) row stride
also avoids SBUF bank conflicts on the DVE reads (a 2KB-stride variant
measured ~20% slower element ops).

Layout: strip layout — partition p owns positions t = p*NT + j, so the +-1
tap shifts are free-dim shifts inside a partition. NT=32 positions split
into 4 tiles of 8 columns.

Engine plan (all engines in parallel, DMA-overlapped):
- SWDGE descriptor gen (~0.8us + ~7.6ns/idx, executed by ONE Q7 core pair
  selected by queue_num; >1024 idx per gather is a hardware crash; the
  sequencer holds each gather until its pair accepts, so queue repeats
  head-of-line block) runs on 4 queues = 4 core pairs. Tile 0 is laddered
  as single columns behind a tiny dummy gather that soaks the first-
  instruction hold, so tile 0 data lands ~5us after the ucode IRAM load.
- DVE op1: part16 = A8' + C8' (int8 inputs run at 1 elem/lane/cycle). Seam
  columns at tile borders are small ops reading the neighbor tile or the
  host-precomputed strip-edge rows; tile 0's op2 runs on cols 0..6 before
  its seamC column so the wait for tile 1's data is filled with work.
- Scalar/Act engine casts the B slice int8->int16 in parallel (primed
  early so ACT_TABLE_LOAD is off the critical path).
- DVE op2: part16 += B16 — all operands 16-bit, step 1 -> DVE 2x mode.
- The last tile's op2+store are split in halves so the final store's DMA
  drain overlaps the last add.
"""

from contextlib import ExitStack

import numpy as np

import concourse.bacc as bacc
import concourse.bass as bass
import concourse.mybir as mybir
import concourse.tile as tile
from concourse._compat import with_exitstack
from concourse.bass_utils import run_bass_kernel_spmd

B = 8
T = 4096
F = 512
V = 32000
VP = V + 1  # +1 zero row
K = 3
P = 128
NT = T // P  # 32 positions per partition strip
NTILE = 4
J = NT // NTILE  # 8 columns per tile
NQ = 4  # SWDGE queues (Q7 core pairs)
BSW = P // 16  # idx slots per gathered column
SW_TOT = NT * BSW  # idx slots per partition
N_CORES = 8
DMA_SCRATCH = 24576

# (tile, col_lo_within_tile, n_cols, queue) in program order; tile 0 single
# columns (first wave q1,q2,q3,q0 behind the dummy that soaks the first
# sequencer hold)
GATHERS = (
    (0, 1, 1, 1), (0, 2, 1, 2), (0, 3, 1, 3), (0, 0, 1, 0),
    (0, 4, 1, 1), (0, 5, 1, 2), (0, 6, 1, 3), (0, 7, 1, 0),
    (1, 0, 4, 1), (1, 4, 4, 2),
    (2, 0, 4, 3), (2, 4, 4, 0),
    (3, 0, 4, 1), (3, 4, 4, 2),
)

_nc_cache = {}


@with_exitstack
def _gather_kernel(ctx: ExitStack, tc: tile.TileContext, out_d, tab_d, idxs_d, bnd_d):
    nc = tc.nc

    idxp = ctx.enter_context(tc.tile_pool(name="idx", bufs=1))
    rp = ctx.enter_context(tc.tile_pool(name="rp", bufs=NTILE))
    pp = ctx.enter_context(tc.tile_pool(name="pp", bufs=NTILE))
    bp = ctx.enter_context(tc.tile_pool(name="bp", bufs=NTILE))
    wp = ctx.enter_context(tc.tile_pool(name="wp", bufs=1))

    idxs_t = idxp.tile([P, SW_TOT], mybir.dt.int16)
    nc.sync.dma_start(idxs_t[:], idxs_d[:])
    bnd_t = idxp.tile([P, 2, F], mybir.dt.int8)
    nc.sync.dma_start(bnd_t[:], bnd_d[:])

    # prime the Act engine's Copy table while the gather ucode loads
    wact = wp.tile([P, 1, 8], mybir.dt.int16)
    nc.scalar.copy(wact[:], bnd_t[:, 0:1, 0:8])

    R = [None] * NTILE
    for t in range(NTILE):
        R[t] = rp.tile([P, J, 3 * F], mybir.dt.int8, tag="R", name=f"R{t}")
    # dummy 16-idx gather: soaks the first-instruction sequencer hold (the
    # first ext-inst occupies the sequencer for its whole generation)
    wdum = wp.tile([P, 1, 3 * F], mybir.dt.int8)
    nc.gpsimd.dma_gather(
        wdum[:], tab_d[:], idxs_t[:, 0:1], 16, 16, 3 * F, queue_num=0
    )
    for t, cl, ncols, q in GATHERS:
        gcol = t * J + cl
        nc.gpsimd.dma_gather(
            R[t][:, cl : cl + ncols, :],
            tab_d[:],
            idxs_t[:, gcol * BSW : (gcol + ncols) * BSW],
            P * ncols,
            P * ncols,
            3 * F,
            queue_num=q,
        )

    def A8(Rt, lo, hi):
        return Rt[:, lo:hi, 0:F]

    def C8(Rt, lo, hi):
        return Rt[:, lo:hi, 2 * F : 3 * F]

    for t in range(NTILE):
        Rt = R[t]
        part = pp.tile([P, J, F], mybir.dt.int16, tag="part", name=f"part{t}")
        b16 = bp.tile([P, J, F], mybir.dt.int16, tag="b16", name=f"b16{t}")
        asrc = bnd_t[:, 0:1, :] if t == 0 else A8(R[t - 1], J - 1, J)
        csrc = bnd_t[:, 1:2, :] if t == NTILE - 1 else C8(R[t + 1], 0, 1)
        if t == 0:
            # tile 0 is laddered column-wise; split ops so DVE/Act start on
            # the first wave (cols 0-3) early, and run op2 on cols 0..J-2
            # before the seamC column (which needs tile 1's data)
            h = J // 2
            nc.scalar.copy(b16[:, 0:h, :], Rt[:, 0:h, F : 2 * F])
            nc.vector.tensor_add(part[:, 0:1, :], asrc, C8(Rt, 1, 2))
            nc.vector.tensor_add(
                part[:, 1 : h - 1, :], A8(Rt, 0, h - 2), C8(Rt, 2, h)
            )
            nc.scalar.copy(b16[:, h:J, :], Rt[:, h:J, F : 2 * F])
            nc.vector.tensor_add(
                part[:, h - 1 : J - 1, :], A8(Rt, h - 2, J - 2), C8(Rt, h, J)
            )
            nc.vector.tensor_add(
                part[:, 0 : J - 1, :],
                part[:, 0 : J - 1, :],
                b16[:, 0 : J - 1, :],
            )
            nc.vector.tensor_add(
                part[:, J - 1 : J, :], A8(Rt, J - 2, J - 1), csrc
            )
            nc.vector.tensor_add(
                part[:, J - 1 : J, :],
                part[:, J - 1 : J, :],
                b16[:, J - 1 : J, :],
            )
            nc.sync.dma_start(out_d[:, t * J : (t + 1) * J, :], part[:])
        else:
            # Act: cast the B slice to int16 while DVE works on A+C
            nc.scalar.copy(b16[:], Rt[:, :, F : 2 * F])
            # DVE op1: part[:, j] = A[:, j-1] + C[:, j+1] (int8+int8 -> int16)
            nc.vector.tensor_add(
                part[:, 1 : J - 1, :], A8(Rt, 0, J - 2), C8(Rt, 2, J)
            )
            nc.vector.tensor_add(part[:, 0:1, :], asrc, C8(Rt, 1, 2))
            nc.vector.tensor_add(
                part[:, J - 1 : J, :], A8(Rt, J - 2, J - 1), csrc
            )
            # DVE op2: += B, all 16-bit -> 2x mode. Last tile split in
            # halves so the final store's DMA drain overlaps the last add.
            if t < NTILE - 1:
                nc.vector.tensor_add(part[:], part[:], b16[:])
                nc.sync.dma_start(out_d[:, t * J : (t + 1) * J, :], part[:])
            else:
                h = J // 2
                nc.vector.tensor_add(
                    part[:, 0:h, :], part[:, 0:h, :], b16[:, 0:h, :]
                )
                nc.sync.dma_start(
                    out_d[:, t * J : t * J + h, :], part[:, 0:h, :]
                )
                nc.vector.tensor_add(
                    part[:, h:J, :], part[:, h:J, :], b16[:, h:J, :]
                )
                nc.sync.dma_start(
                    out_d[:, t * J + h : (t + 1) * J, :], part[:, h:J, :]
                )


def _build_nc():
    if "nc" in _nc_cache:
        return _nc_cache["nc"]
    nc = bacc.Bacc(
        "TRN2",
        target_bir_lowering=False,
        debug=False,
        enable_asserts=False,
        num_devices=N_CORES,
        dynamic_dma_scratch_size=DMA_SCRATCH,
        num_swdge_queues=NQ,
    )
    tab_d = nc.dram_tensor(
        "tab", [VP, 3 * F], mybir.dt.int8, kind="ExternalInput"
    ).ap()
    idxs_d = nc.dram_tensor(
        "idxs", [P, SW_TOT], mybir.dt.int16, kind="ExternalInput"
    ).ap()
    bnd_d = nc.dram_tensor(
        "bnd", [P, 2, F], mybir.dt.int8, kind="ExternalInput"
    ).ap()
    out_d = nc.dram_tensor(
        "out", [P, NT, F], mybir.dt.int16, kind="ExternalOutput"
    ).ap()
    with tile.TileContext(nc) as tc:
        _gather_kernel(tc, out_d, tab_d, idxs_d, bnd_d)
    nc.compile()
    _nc_cache["nc"] = nc
    return nc


def _wrap16(stream):
    # gather idx wrap: idx i read from partition i%16, slot i//16; x8 replicas
    n = stream.shape[-1]
    w = stream.reshape(*stream.shape[:-1], n // 16, 16)
    w = np.swapaxes(w, -1, -2)  # [..., 16, n//16]
    reps = [1] * (w.ndim - 2) + [8, 1]
    return np.tile(w, reps)  # [..., 128, n//16]


def _host_prep(token_ids, conv_w):
    # TAB[v] = [A|B|C]: TAB[v, k*F+f] ~ conv_w[f, v, k] / step, int8
    w = np.asarray(conv_w, dtype=np.float32)
    step = (float(np.abs(w).max()) / 127.0) or 1.0
    tab = np.empty((VP, K * F), dtype=np.int8)
    q = np.rint(w.transpose(1, 2, 0).reshape(V, K * F) * (1.0 / step))
    tab[:V] = np.clip(q, -127, 127).astype(np.int8)
    tab[V] = 0

    tok = np.asarray(token_ids).astype(np.int16)  # V=32000 fits int16
    strip = tok.reshape(B, P, NT)

    # fused streams: per gather, stream[g*128 + p] = strip[b, p, col0+g]
    idxs = np.empty((B, P, SW_TOT), dtype=np.int16)
    for t, cl, ncols, _ in GATHERS:
        gcol = t * J + cl
        x = strip[:, :, gcol : gcol + ncols]  # [b, p, g]
        stream = np.ascontiguousarray(x.transpose(0, 2, 1)).reshape(B, ncols * P)
        idxs[:, :, gcol * BSW : (gcol + ncols) * BSW] = _wrap16(stream)

    # strip-edge rows, host-gathered: bnd[:, 0] = A[tok[p*NT-1]] (zeros at
    # p=0), bnd[:, 1] = C[tok[p*NT+NT]] (zeros at p=127)
    bnd = np.zeros((B, P, 2, F), dtype=np.int8)
    bnd[:, 1:, 0] = tab[:, 0:F][strip[:, :-1, NT - 1]]
    bnd[:, : P - 1, 1] = tab[:, 2 * F : 3 * F][strip[:, 1:, 0]]
    return tab, np.ascontiguousarray(idxs), bnd, step


def kernel(token_ids, conv_w):
    tab, idxs, bnd, step = _host_prep(token_ids, conv_w)
    nc = _build_nc()
    in_maps = [
        {"tab": tab, "idxs": idxs[b], "bnd": bnd[b]} for b in range(B)
    ]
    res = run_bass_kernel_spmd(nc, in_maps, core_ids=list(range(N_CORES)))
    # [P, NT, F] with t = p*NT + j flattens directly to [T, F]
    out = np.stack(
        [res.results[b]["out"].reshape(T, F).astype(np.float32) for b in range(B)],
        axis=0,
    )
    out *= np.float32(step)
    return np.ascontiguousarray(out)
